# revision 2
# baseline (speedup 1.0000x reference)
"""Trainium2 Bass kernel for nn_DistanceModule.

Computes, for h [4,512,64], W [64,64], b/gamma/beta [64]:
    x = LayerNorm(ReLU(h @ W.T + b))          # [B,N,C]
    D[b,i,j,c] = x[b,i,c] * x[b,j,c]
    out = softmax(D, axis=-1)                 # [B,N,N,C] f32 (256 MB)

Sharding: 2048 (b,i) rows split across 8 cores -> 256 rows/core
(core k: batch b=k//2, i-half k%2). Each core receives h[b] with rows
rolled so its own i-rows come first; the host un-rolls the j axis of
the output. All cores run one identical NEFF.

Per-core pipeline (all engines overlapped):
  PE     : fp16 product matmuls compute the FULL logit block
           x_i[c]*x_j[c] directly into PSUM. lhsT = xT_f16 (c on
           partitions, K=64); rhs = a "diagonalized" xT tensor diag
           [64, 64*512] fp16 where partition c holds xT[c,:] in column
           block c (so column (c,j) of the matmul output is
           x_i[c]*x[j,c]). fp16 rounding of x costs ~5e-4 rel_fro.
  ScalarE: one wide exp activation per PSUM half (FD=2048, PSUM src),
           no per-channel scale needed -> ~16x less instruction
           overhead than per-channel activations. Output bf16 to SBUF
           with a (c,j)->(j,c) strided AP, giving the store layout.
  VectorE: channel sums via a 6-level bf16 add-tree (tensor_tensor at
           2x 16-bit mode ~ 2x faster than the 1x-only tensor_reduce),
           reciprocal_approx_fast, then one in-place bf16 normalize
           multiply against a pair-duplicated reciprocal AP.
  DMA    : bf16 stores (halves HBM traffic vs f32; softmax outputs in
           [0,1] keep rel_fro ~1.4e-3). Host casts back to f32.

Softmax needs no max-subtraction: LayerNorm bounds |x| by sqrt(C-1),
logits <= 63, exp <= 2.4e27 which fits bf16 range.
Predicted rel err ~4e-3 vs f32 reference (gate 2e-2).
"""

import numpy as np

import concourse.bacc as bacc
import concourse.bass as bass
import concourse.mybir as mybir
import concourse.tile as tile
from concourse.bass_utils import run_bass_kernel_spmd

B, N, C = 4, 512, 64
NCORES = 8
ROWS = 256          # (b,i) rows per core
EPS = 1e-5
F32 = mybir.dt.float32
BF16 = mybir.dt.bfloat16
FP16 = mybir.dt.float16

_CACHE = {}


def _build_program():
    nc = bacc.Bacc(
        "TRN2",
        target_bir_lowering=False,
        debug=False,
        enable_asserts=False,
        num_devices=NCORES,
    )

    hT_d = nc.dram_tensor("hT", [C, N], F32, kind="ExternalInput")
    WT_d = nc.dram_tensor("WT", [C, C], F32, kind="ExternalInput")
    bgb_d = nc.dram_tensor("bgb", [128, 3 * C], F32, kind="ExternalInput")
    id_d = nc.dram_tensor("identity", [128, 128], F32, kind="ExternalInput")
    out_d = nc.dram_tensor("out", [ROWS, N * C], BF16, kind="ExternalOutput")

    sub = mybir.AluOpType.subtract
    mult = mybir.AluOpType.mult
    Exp = mybir.ActivationFunctionType.Exp
    Ln = mybir.ActivationFunctionType.Ln
    Relu = mybir.ActivationFunctionType.Relu

    with tile.TileContext(nc) as tc:
        with tc.tile_pool(name="const", bufs=1) as constp:
            hT = constp.tile([C, N], F32)
            nc.sync.dma_start(hT[:], hT_d[:])
            WT = constp.tile([C, C], F32)
            nc.sync.dma_start(WT[:], WT_d[:])
            bgb = constp.tile([128, 3 * C], F32)
            nc.sync.dma_start(bgb[:], bgb_d[:])
            ident = constp.tile([128, 128], F32)
            nc.sync.dma_start(ident[:], id_d[:])

            # x transposed (c on partitions), fp16, for matmul operands
            xT = constp.tile([C, N], FP16)
            # diagonalized xT: partition c holds xT[c, :] at columns
            # [c*N, (c+1)*N); all other columns zero.
            diag = constp.tile([C, C * N], FP16)
            nc.vector.memset(diag[:], 0.0)
            eps_t = constp.tile([128, 1], F32)
            nc.vector.memset(eps_t[:], EPS)

            # ---- x = LayerNorm(ReLU(h @ W.T + b)), transposed to fp16 ----
            # rstd = exp(-0.5*ln(var+eps)) keeps ScalarE inside the
            # natural_log_exp table set (no Sqrt set-switch).
            with (
                tc.tile_pool(name="xprep", bufs=2) as xprep,
                tc.tile_pool(name="psum_prep", bufs=2, space=bass.MemorySpace.PSUM) as psp,
            ):
                for t in range(4):
                    xp = psp.tile([128, C], F32, tag="xp")
                    nc.tensor.matmul(xp[:], hT[:, t * 128:(t + 1) * 128], WT[:])
                    xs = xprep.tile([128, C], F32, tag="xs")
                    nc.vector.tensor_add(xs[:], xp[:], bgb[:, 0:C])      # + b
                    nc.scalar.activation(xs[:], xs[:], Relu)
                    stats = xprep.tile([128, 6], F32, tag="stats")
                    nc.vector.bn_stats(stats[:], xs[:])
                    mv = xprep.tile([128, 2], F32, tag="mv")
                    nc.vector.bn_aggr(mv[:], stats[:])
                    lnv = xprep.tile([128, 1], F32, tag="lnv")
                    nc.scalar.activation(lnv[:], mv[:, 1:2], Ln, bias=eps_t[:, 0:1])
                    rstd = xprep.tile([128, 1], F32, tag="rstd")
                    nc.scalar.activation(rstd[:], lnv[:], Exp, scale=-0.5)
                    xn = xprep.tile([128, C], F32, tag="xn")
                    nc.vector.tensor_scalar(
                        xn[:], xs[:], mv[:, 0:1], rstd[:, 0:1], op0=sub, op1=mult
                    )
                    nc.vector.tensor_mul(xn[:], xn[:], bgb[:, C:2 * C])  # * gamma
                    nc.vector.tensor_add(xn[:], xn[:], bgb[:, 2 * C:3 * C])  # + beta
                    tp = psp.tile([C, 128], F32, tag="tp")
                    nc.tensor.transpose(tp[:], xn[:], ident[:])
                    nc.vector.tensor_copy(xT[:, t * 128:(t + 1) * 128], tp[:])

            # place xT rows on the diagonal blocks (64 tiny SBUF->SBUF DMAs,
            # one-time; each is a contiguous 1KB row copy)
            for c in range(C):
                nc.sync.dma_start(diag[c:c + 1, c * N:(c + 1) * N], xT[c:c + 1, :])

            # ---- main: exp(x_i*x_j), softmax over c, store ----------------
            # unit = (i-tile, j-half) = [128 i, 256 j, 64 c]
            # per unit: 8 PSUM rounds of 8 channels; each round = 8 fp16
            # matmuls [64,256] + 1 wide exp activation (FD=2048).
            JW = 256
            CB = 8            # channels per PSUM round
            with (
                tc.tile_pool(name="main", bufs=2) as mainp,
                tc.tile_pool(name="scr", bufs=2) as scrp,
                tc.tile_pool(name="small", bufs=3) as smallp,
                tc.tile_pool(name="psum_bc", bufs=2, space=bass.MemorySpace.PSUM) as pbc,
            ):
                for it in range(2):
                    lhsT = xT[:, it * 128:(it + 1) * 128]
                    for jh in range(2):
                        j0 = jh * JW
                        expt = mainp.tile([128, JW, C], BF16, tag="exp")
                        for r in range(C // CB):
                            ps = pbc.tile([128, CB, JW], F32, tag="ps")
                            for ci in range(CB):
                                c = r * CB + ci
                                nc.tensor.matmul(
                                    ps[:, ci, :],
                                    lhsT,
                                    diag[:, c * N + j0:c * N + j0 + JW],
                                )
                            nc.scalar.activation(
                                expt[:, :, r * CB:(r + 1) * CB].rearrange(
                                    "p j c -> p c j"
                                ),
                                ps[:],
                                Exp,
                            )
                        # channel sums: 6-level bf16 add-tree (2x mode)
                        sc = scrp.tile([128, JW, 32], BF16, tag="sc")
                        nc.vector.tensor_add(
                            sc[:], expt[:, :, 0:32], expt[:, :, 32:64]
                        )
                        w = 16
                        while w >= 1:
                            nc.vector.tensor_add(
                                sc[:, :, 0:w], sc[:, :, 0:w], sc[:, :, w:2 * w]
                            )
                            w //= 2
                        s32 = smallp.tile([128, JW], F32, tag="s32")
                        nc.vector.tensor_copy(s32[:], sc[:, :, 0])
                        recip = smallp.tile([128, JW], F32, tag="recip")
                        nc.vector.reciprocal_approx_fast(recip[:], s32[:])
                        # pair-duplicated bf16 reciprocal for the 2x-mode
                        # broadcast multiply
                        rp = smallp.tile([128, JW, 2], BF16, tag="rp")
                        nc.vector.tensor_copy(
                            rp[:], recip[:, :, None].broadcast_to((128, JW, 2))
                        )
                        # normalize in j-quarters; each quarter DMAs out as
                        # soon as it is scaled
                        QW = JW // 4
                        for q in range(4):
                            sl = slice(q * QW, (q + 1) * QW)
                            nc.vector.tensor_mul(
                                expt[:, sl, :].rearrange("p j (a b) -> p j a b", b=2),
                                expt[:, sl, :].rearrange("p j (a b) -> p j a b", b=2),
                                rp[:, sl, None, :].broadcast_to((128, QW, 32, 2)),
                            )
                            nc.sync.dma_start(
                                out_d[it * 128:(it + 1) * 128,
                                      (j0 + q * QW) * C:(j0 + (q + 1) * QW) * C],
                                expt[:, sl, :].rearrange("p j c -> p (j c)"),
                            )
    nc.compile()
    return nc


def _in_maps(h, W, b, gamma, beta):
    h = np.asarray(h, dtype=np.float32)
    W = np.asarray(W, dtype=np.float32)
    b = np.asarray(b, dtype=np.float32)
    gamma = np.asarray(gamma, dtype=np.float32)
    beta = np.asarray(beta, dtype=np.float32)

    WT = np.ascontiguousarray(W.T)
    bgb = np.ascontiguousarray(
        np.broadcast_to(np.concatenate([b, gamma, beta])[None, :], (128, 3 * C))
    )
    ident = np.eye(128, dtype=np.float32)

    in_maps = []
    for k in range(NCORES):
        bb, half = divmod(k, 2)
        i0 = half * ROWS
        # roll rows so this core's i-rows come first; host un-rolls j
        hloc = np.roll(h[bb], -i0, axis=0)
        in_maps.append({
            "hT": np.ascontiguousarray(hloc.T),
            "WT": WT,
            "bgb": bgb,
            "identity": ident,
        })
    return in_maps


def run(h, W, b, gamma, beta, trace=False, **trace_kwargs):
    if "nc" not in _CACHE:
        _CACHE["nc"] = _build_program()
    nc = _CACHE["nc"]
    res = run_bass_kernel_spmd(
        nc,
        _in_maps(h, W, b, gamma, beta),
        core_ids=list(range(NCORES)),
        trace=trace,
        **trace_kwargs,
    )
    out = np.zeros((B, N, N, C), dtype=np.float32)
    for k in range(NCORES):
        bb, half = divmod(k, 2)
        i0 = half * ROWS
        buf = np.asarray(res.results[k]["out"]).astype(np.float32)
        buf = buf.reshape(ROWS, N, C)
        out[bb, i0:i0 + ROWS] = np.roll(buf, i0, axis=1)
    return out, res


def kernel(h, W, b, gamma, beta):
    out, _ = run(h, W, b, gamma, beta)
    return out


# revision 3
# speedup vs baseline: 2.1239x; 2.1239x over previous
"""Trainium2 Bass kernel for nn_DistanceModule.

Computes, for h [4,512,64], W [64,64], b/gamma/beta [64]:
    x = LayerNorm(ReLU(h @ W.T + b))          # [B,N,C]
    D[b,i,j,c] = x[b,i,c] * x[b,j,c]
    out = softmax(D, axis=-1)                 # [B,N,N,C] f32 (256 MB)

Sharding: 2048 (b,i) rows split across 8 cores -> 256 rows/core
(core k: batch b=k//2, i-half k%2). Each core receives h[b] with rows
rolled so its own i-rows come first; the host un-rolls the j axis of
the output. All cores run one identical NEFF.

Per-core pipeline, c-major on-chip layout (contiguous APs everywhere):
  PE     : fp16 product matmuls compute the FULL logit block
           x_i[c]*x_j[c] directly into PSUM. lhsT = xT_f16 (c on
           partitions, K=64); rhs = a "diagonalized" xT tensor diag
           [64, 64*512] fp16 where partition c holds xT[c,:] in column
           block c, so matmul column (c,j) yields x_i[c]*x[j,c].
           Dense back-to-back matmuls keep the PE HAM clock-gate warm.
  ScalarE: one contiguous exp activation per PSUM half (FD=2048, PSUM
           src, bf16 dst) -- no per-channel scale, no strided writes.
           Only Ln/Exp are used on ScalarE (single table set).
  VectorE: channel sums via a bf16 add-tree over contiguous c-halves
           (tensor_tensor 2x 16-bit mode beats the 1x-only
           tensor_reduce), reciprocal_approx_fast, then an in-place
           bf16 normalize multiply against a stride-0-broadcast
           reciprocal (innermost step 1 keeps 2x mode).
  DMA    : contiguous bf16 stores in (c,j) order; the host transposes
           each [64,256] block to (j,c) while casting back to f32.

Softmax needs no max-subtraction: LayerNorm bounds |x| by sqrt(C-1),
logits <= 63, exp <= 2.4e27 which fits bf16 range.
Measured rel err ~4e-3 vs f32 reference (harness gate 2e-2).
"""

import numpy as np

import concourse.bacc as bacc
import concourse.bass as bass
import concourse.mybir as mybir
import concourse.tile as tile
from concourse.bass_utils import run_bass_kernel_spmd

B, N, C = 4, 512, 64
NCORES = 8
ROWS = 256          # (b,i) rows per core
JW = 256            # j-half width
CB = 8              # channels per PSUM round
EPS = 1e-5
F32 = mybir.dt.float32
BF16 = mybir.dt.bfloat16
FP16 = mybir.dt.float16

_CACHE = {}


def _build_program():
    nc = bacc.Bacc(
        "TRN2",
        target_bir_lowering=False,
        debug=False,
        enable_asserts=False,
        num_devices=NCORES,
    )

    hT_d = nc.dram_tensor("hT", [C, N], F32, kind="ExternalInput")
    WT_d = nc.dram_tensor("WT", [C, C], F32, kind="ExternalInput")
    bgb_d = nc.dram_tensor("bgb", [128, 3 * C], F32, kind="ExternalInput")
    id_d = nc.dram_tensor("identity", [128, 128], F32, kind="ExternalInput")
    # (c,j)-major per (i,jh): [i, jh*C*JW + c*JW + j]
    out_d = nc.dram_tensor("out", [ROWS, N * C], BF16, kind="ExternalOutput")

    sub = mybir.AluOpType.subtract
    mult = mybir.AluOpType.mult
    Exp = mybir.ActivationFunctionType.Exp
    Ln = mybir.ActivationFunctionType.Ln

    with tile.TileContext(nc) as tc:
        with tc.tile_pool(name="const", bufs=1) as constp:
            hT = constp.tile([C, N], F32)
            nc.sync.dma_start(hT[:], hT_d[:])
            WT = constp.tile([C, C], F32)
            nc.sync.dma_start(WT[:], WT_d[:])
            bgb = constp.tile([128, 3 * C], F32)
            nc.sync.dma_start(bgb[:], bgb_d[:])
            ident = constp.tile([128, 128], F32)
            nc.sync.dma_start(ident[:], id_d[:])

            # x transposed (c on partitions), fp16, for matmul operands
            xT = constp.tile([C, N], FP16)
            # diagonalized xT: partition c holds xT[c, :] at columns
            # [c*N, (c+1)*N); all other columns zero.
            diag = constp.tile([C, C * N], FP16)
            nc.gpsimd.memset(diag[:], 0.0)
            eps_t = constp.tile([128, 1], F32)
            nc.vector.memset(eps_t[:], EPS)

            # ---- x = LayerNorm(ReLU(h @ W.T + b)), transposed to fp16 ----
            # ReLU on VectorE and rstd = exp(-0.5*ln(var+eps)) keep ScalarE
            # inside the natural_log_exp table set (one ACT_TABLE_LOAD).
            with (
                tc.tile_pool(name="xprep", bufs=2) as xprep,
                tc.tile_pool(name="psum_prep", bufs=2, space=bass.MemorySpace.PSUM) as psp,
            ):
                for t in range(4):
                    xp = psp.tile([128, C], F32, tag="xp")
                    nc.tensor.matmul(xp[:], hT[:, t * 128:(t + 1) * 128], WT[:])
                    xs = xprep.tile([128, C], F32, tag="xs")
                    nc.vector.tensor_add(xs[:], xp[:], bgb[:, 0:C])      # + b
                    nc.vector.tensor_scalar_max(xs[:], xs[:], 0.0)       # ReLU
                    stats = xprep.tile([128, 6], F32, tag="stats")
                    nc.vector.bn_stats(stats[:], xs[:])
                    mv = xprep.tile([128, 2], F32, tag="mv")
                    nc.vector.bn_aggr(mv[:], stats[:])
                    lnv = xprep.tile([128, 1], F32, tag="lnv")
                    nc.scalar.activation(lnv[:], mv[:, 1:2], Ln, bias=eps_t[:, 0:1])
                    rstd = xprep.tile([128, 1], F32, tag="rstd")
                    nc.scalar.activation(rstd[:], lnv[:], Exp, scale=-0.5)
                    xn = xprep.tile([128, C], F32, tag="xn")
                    nc.vector.tensor_scalar(
                        xn[:], xs[:], mv[:, 0:1], rstd[:, 0:1], op0=sub, op1=mult
                    )
                    nc.vector.tensor_mul(xn[:], xn[:], bgb[:, C:2 * C])  # * gamma
                    nc.vector.tensor_add(xn[:], xn[:], bgb[:, 2 * C:3 * C])  # + beta
                    tp = psp.tile([C, 128], F32, tag="tp")
                    nc.tensor.transpose(tp[:], xn[:], ident[:])
                    nc.vector.tensor_copy(xT[:, t * 128:(t + 1) * 128], tp[:])

            # place xT rows on the diagonal blocks (64 tiny SBUF->SBUF DMAs,
            # one-time; each a contiguous 1KB row copy)
            for c in range(C):
                nc.sync.dma_start(diag[c:c + 1, c * N:(c + 1) * N], xT[c:c + 1, :])

            # ---- main: exp(x_i*x_j), softmax over c, store ----------------
            # unit = (i-tile, j-half): expt [128, 64c, 256j] bf16 (c-major).
            # per unit: 8 PSUM rounds; each = 8 fp16 matmuls [64,256] into
            # one PSUM half + 1 contiguous exp activation (FD=2048).
            with (
                tc.tile_pool(name="main", bufs=2) as mainp,
                tc.tile_pool(name="scr", bufs=2) as scrp,
                tc.tile_pool(name="small", bufs=3) as smallp,
                tc.tile_pool(name="psum_bc", bufs=2, space=bass.MemorySpace.PSUM) as pbc,
            ):
                for it in range(2):
                    lhsT = xT[:, it * 128:(it + 1) * 128]
                    for jh in range(2):
                        j0 = jh * JW
                        expt = mainp.tile([128, C, JW], BF16, tag="exp")
                        for r in range(C // CB):
                            ps = pbc.tile([128, CB, JW], F32, tag="ps")
                            for ci in range(CB):
                                c = r * CB + ci
                                nc.tensor.matmul(
                                    ps[:, ci, :],
                                    lhsT,
                                    diag[:, c * N + j0:c * N + j0 + JW],
                                )
                            nc.scalar.activation(
                                expt[:, r * CB:(r + 1) * CB, :], ps[:], Exp
                            )
                        # channel sums: 6-level bf16 add-tree over contiguous
                        # c-halves (2x mode)
                        sc = scrp.tile([128, 32, JW], BF16, tag="sc")
                        nc.vector.tensor_add(sc[:], expt[:, 0:32, :], expt[:, 32:64, :])
                        w = 16
                        while w >= 2:
                            nc.vector.tensor_add(
                                sc[:, 0:w, :], sc[:, 0:w, :], sc[:, w:2 * w, :]
                            )
                            w //= 2
                        s32 = smallp.tile([128, JW], F32, tag="s32")
                        nc.vector.tensor_add(s32[:], sc[:, 0, :], sc[:, 1, :])
                        recip = smallp.tile([128, JW], F32, tag="recip")
                        nc.vector.reciprocal_approx_fast(recip[:], s32[:])
                        rb = smallp.tile([128, JW], BF16, tag="rb")
                        nc.vector.tensor_copy(rb[:], recip[:])
                        # normalize in c-chunks; each chunk DMAs out as soon
                        # as it is scaled
                        CC = 16
                        for q in range(C // CC):
                            cs = slice(q * CC, (q + 1) * CC)
                            nc.vector.tensor_mul(
                                expt[:, cs, :],
                                expt[:, cs, :],
                                rb[:, None, :].broadcast_to((128, CC, JW)),
                            )
                            nc.sync.dma_start(
                                out_d[it * 128:(it + 1) * 128,
                                      jh * C * JW + q * CC * JW:
                                      jh * C * JW + (q + 1) * CC * JW],
                                expt[:, cs, :].rearrange("p c j -> p (c j)"),
                            )
    nc.compile()
    return nc


def _in_maps(h, W, b, gamma, beta):
    h = np.asarray(h, dtype=np.float32)
    W = np.asarray(W, dtype=np.float32)
    b = np.asarray(b, dtype=np.float32)
    gamma = np.asarray(gamma, dtype=np.float32)
    beta = np.asarray(beta, dtype=np.float32)

    WT = np.ascontiguousarray(W.T)
    bgb = np.ascontiguousarray(
        np.broadcast_to(np.concatenate([b, gamma, beta])[None, :], (128, 3 * C))
    )
    ident = np.eye(128, dtype=np.float32)

    in_maps = []
    for k in range(NCORES):
        bb, half = divmod(k, 2)
        i0 = half * ROWS
        # roll rows so this core's i-rows come first; host un-rolls j
        hloc = np.roll(h[bb], -i0, axis=0)
        in_maps.append({
            "hT": np.ascontiguousarray(hloc.T),
            "WT": WT,
            "bgb": bgb,
            "identity": ident,
        })
    return in_maps


def run(h, W, b, gamma, beta, trace=False, **trace_kwargs):
    if "nc" not in _CACHE:
        _CACHE["nc"] = _build_program()
    nc = _CACHE["nc"]
    res = run_bass_kernel_spmd(
        nc,
        _in_maps(h, W, b, gamma, beta),
        core_ids=list(range(NCORES)),
        trace=trace,
        **trace_kwargs,
    )
    out = np.zeros((B, N, N, C), dtype=np.float32)
    for k in range(NCORES):
        bb, half = divmod(k, 2)
        i0 = half * ROWS
        buf = np.asarray(res.results[k]["out"]).astype(np.float32)
        # [ROWS, jh, c, j] -> [ROWS, (jh j), c], then un-roll j
        buf = buf.reshape(ROWS, 2, C, JW).transpose(0, 1, 3, 2).reshape(ROWS, N, C)
        out[bb, i0:i0 + ROWS] = np.roll(buf, i0, axis=1)
    return out, res


def kernel(h, W, b, gamma, beta):
    out, _ = run(h, W, b, gamma, beta)
    return out


# revision 10
# speedup vs baseline: 2.3931x; 1.1267x over previous
"""Trainium2 Bass kernel for nn_DistanceModule.

Computes, for h [4,512,64], W [64,64], b/gamma/beta [64]:
    x = LayerNorm(ReLU(h @ W.T + b))          # [B,N,C]
    D[b,i,j,c] = x[b,i,c] * x[b,j,c]
    out = softmax(D, axis=-1)                 # [B,N,N,C] f32 (256 MB)

Sharding: 2048 (b,i) rows split across 8 cores -> 256 rows/core
(core k: batch b=k//2, i-half k%2). Each core receives h[b] with rows
rolled so its own i-rows come first; the host un-rolls the j axis of
the output. All cores run one identical NEFF.

Per-core pipeline, c-major on-chip layout (contiguous APs everywhere):
  PE     : fp16 product matmuls compute the FULL logit block
           x_i[c]*x_j[c] directly into PSUM. lhsT = xT_f16 (c on
           partitions, K=64); rhs = a "diagonalized" xT tensor diag
           [64, 64*512] fp16 where partition c holds xT[c,:] in column
           block c, so matmul column (c,j) yields x_i[c]*x[j,c].
           Dense back-to-back matmuls keep the PE HAM clock-gate warm.
  ScalarE: one contiguous exp activation per PSUM half (FD=2048, PSUM
           src, bf16 dst) -- no per-channel scale, no strided writes.
           Only Ln/Exp are used on ScalarE (single table set).
  VectorE: channel sums via a bf16 add-tree over contiguous c-halves
           (tensor_tensor 2x 16-bit mode beats the 1x-only
           tensor_reduce), reciprocal_approx_fast, then an in-place
           bf16 normalize multiply against a stride-0-broadcast
           reciprocal (innermost step 1 keeps 2x mode).
  DMA    : contiguous bf16 stores in (c,j) order; the host transposes
           each [64,256] block to (j,c) while casting back to f32.

Softmax needs no max-subtraction: LayerNorm bounds |x| by sqrt(C-1),
logits <= 63, exp <= 2.4e27 which fits bf16 range.
Measured rel err ~4e-3 vs f32 reference (harness gate 2e-2).
"""

import numpy as np

import concourse.bacc as bacc
import concourse.bass as bass
import concourse.mybir as mybir
import concourse.tile as tile
from concourse.bass_utils import run_bass_kernel_spmd

B, N, C = 4, 512, 64
NCORES = 8
ROWS = 256          # (b,i) rows per core
JW = 256            # j-half width
CB = 8              # channels per PSUM round
EPS = 1e-5
F32 = mybir.dt.float32
BF16 = mybir.dt.bfloat16
FP16 = mybir.dt.float16

_CACHE = {}


def _build_program():
    nc = bacc.Bacc(
        "TRN2",
        target_bir_lowering=False,
        debug=False,
        enable_asserts=False,
        num_devices=NCORES,
    )

    hT_d = nc.dram_tensor("hT", [C, N], F32, kind="ExternalInput")
    WT_d = nc.dram_tensor("WT", [C, C], F32, kind="ExternalInput")
    bgb_d = nc.dram_tensor("bgb", [128, 3 * C], F32, kind="ExternalInput")
    id_d = nc.dram_tensor("identity", [128, 128], F32, kind="ExternalInput")
    xstage_d = nc.dram_tensor("xstage", [C, N], FP16, kind="Internal")
    # (c,j)-major per (i,jh): [i, jh*C*JW + c*JW + j]
    out_d = nc.dram_tensor("out", [ROWS, N * C], BF16, kind="ExternalOutput")

    sub = mybir.AluOpType.subtract
    mult = mybir.AluOpType.mult
    Exp = mybir.ActivationFunctionType.Exp
    Ln = mybir.ActivationFunctionType.Ln

    with tile.TileContext(nc) as tc:
        with tc.tile_pool(name="const", bufs=1) as constp:
            hT = constp.tile([C, N], F32)
            nc.sync.dma_start(hT[:], hT_d[:])
            WT = constp.tile([C, C], F32)
            nc.sync.dma_start(WT[:], WT_d[:])
            bgb = constp.tile([128, 3 * C], F32)
            nc.sync.dma_start(bgb[:], bgb_d[:])
            ident = constp.tile([128, 128], F32)
            nc.sync.dma_start(ident[:], id_d[:])

            # x transposed (c on partitions), fp16, for matmul operands
            xT = constp.tile([C, N], FP16)
            eps_t = constp.tile([128, 1], F32)
            nc.vector.memset(eps_t[:], EPS)

            # ---- x = LayerNorm(ReLU(h @ W.T + b)), transposed to fp16 ----
            # ReLU on VectorE and rstd = exp(-0.5*ln(var+eps)) keep ScalarE
            # inside the natural_log_exp table set (one ACT_TABLE_LOAD).
            with (
                tc.tile_pool(name="xprep", bufs=2) as xprep,
                tc.tile_pool(name="psum_prep", bufs=2, space=bass.MemorySpace.PSUM) as psp,
            ):
                for t in range(4):
                    xp = psp.tile([128, C], F32, tag="xp")
                    nc.tensor.matmul(xp[:], hT[:, t * 128:(t + 1) * 128], WT[:])
                    xs = xprep.tile([128, C], F32, tag="xs")
                    nc.vector.tensor_add(xs[:], xp[:], bgb[:, 0:C])      # + b
                    nc.vector.tensor_scalar_max(xs[:], xs[:], 0.0)       # ReLU
                    stats = xprep.tile([128, 6], F32, tag="stats")
                    nc.vector.bn_stats(stats[:], xs[:])
                    mv = xprep.tile([128, 2], F32, tag="mv")
                    nc.vector.bn_aggr(mv[:], stats[:])
                    lnv = xprep.tile([128, 1], F32, tag="lnv")
                    nc.scalar.activation(lnv[:], mv[:, 1:2], Ln, bias=eps_t[:, 0:1])
                    rstd = xprep.tile([128, 1], F32, tag="rstd")
                    nc.scalar.activation(rstd[:], lnv[:], Exp, scale=-0.5)
                    xn = xprep.tile([128, C], F32, tag="xn")
                    nc.vector.tensor_scalar(
                        xn[:], xs[:], mv[:, 0:1], rstd[:, 0:1], op0=sub, op1=mult
                    )
                    nc.vector.tensor_mul(xn[:], xn[:], bgb[:, C:2 * C])  # * gamma
                    nc.vector.tensor_add(xn[:], xn[:], bgb[:, 2 * C:3 * C])  # + beta
                    tp = psp.tile([C, 128], F32, tag="tp")
                    nc.tensor.transpose(tp[:], xn[:], ident[:])
                    nc.vector.tensor_copy(xT[:, t * 128:(t + 1) * 128], tp[:])

            # concatenate all xT rows onto partition 0 (matmul operands must
            # have base partition 0/32/64): SBUF -> DRAM -> SBUF round-trip
            xTcat = constp.tile([1, C * N], FP16)
            nc.sync.dma_start(xstage_d[:], xT[:])
            nc.sync.dma_start(
                xTcat[0:1, :], xstage_d[:].rearrange("a b -> (a b)")[None, :]
            )

            # ---- main: exp(x_i*x_j), softmax over c, store ----------------
            # unit = (i-tile, j-half): expt [128, 64c, 256j] bf16 (c-major).
            # per unit: 8 PSUM rounds; each = 8 K=1 outer-product fp16
            # matmuls (lhsT = xTcat[0, c*N+i-tile] [1,128], rhs =
            # xTcat[0, c*N+j-half] [1,256] -> out[i,j] = x_i[c]*x_j[c])
            # + 1 contiguous exp activation (FD=2048).
            with (
                tc.tile_pool(name="main", bufs=2) as mainp,
                tc.tile_pool(name="scr", bufs=2) as scrp,
                tc.tile_pool(name="small", bufs=3) as smallp,
                tc.tile_pool(name="psum_bc", bufs=2, space=bass.MemorySpace.PSUM) as pbc,
            ):
                for it in range(2):
                    i0 = it * 128
                    for jh in range(2):
                        j0 = jh * JW
                        expt = mainp.tile([128, C, JW], BF16, tag="exp")
                        for r in range(C // CB):
                            ps = pbc.tile([128, CB, JW], F32, tag="ps")
                            for ci in range(CB):
                                c = r * CB + ci
                                nc.tensor.matmul(
                                    ps[:, ci, :],
                                    xTcat[0:1, c * N + i0:c * N + i0 + 128],
                                    xTcat[0:1, c * N + j0:c * N + j0 + JW],
                                )
                            nc.scalar.activation(
                                expt[:, r * CB:(r + 1) * CB, :], ps[:], Exp
                            )
                        # channel sums: 6-level bf16 add-tree over contiguous
                        # c-halves (2x mode)
                        sc = scrp.tile([128, 32, JW], BF16, tag="sc")
                        nc.vector.tensor_add(sc[:], expt[:, 0:32, :], expt[:, 32:64, :])
                        w = 16
                        while w >= 2:
                            nc.vector.tensor_add(
                                sc[:, 0:w, :], sc[:, 0:w, :], sc[:, w:2 * w, :]
                            )
                            w //= 2
                        s32 = smallp.tile([128, JW], F32, tag="s32")
                        nc.vector.tensor_add(s32[:], sc[:, 0, :], sc[:, 1, :])
                        recip = smallp.tile([128, JW], F32, tag="recip")
                        nc.vector.reciprocal_approx_fast(recip[:], s32[:])
                        rb = smallp.tile([128, JW], BF16, tag="rb")
                        nc.vector.tensor_copy(rb[:], recip[:])
                        # normalize in c-chunks; each chunk DMAs out as soon
                        # as it is scaled
                        CC = 16
                        for q in range(C // CC):
                            cs = slice(q * CC, (q + 1) * CC)
                            nc.vector.tensor_mul(
                                expt[:, cs, :],
                                expt[:, cs, :],
                                rb[:, None, :].broadcast_to((128, CC, JW)),
                            )
                            nc.sync.dma_start(
                                out_d[it * 128:(it + 1) * 128,
                                      jh * C * JW + q * CC * JW:
                                      jh * C * JW + (q + 1) * CC * JW],
                                expt[:, cs, :].rearrange("p c j -> p (c j)"),
                            )
    nc.compile()
    return nc


def _in_maps(h, W, b, gamma, beta):
    h = np.asarray(h, dtype=np.float32)
    W = np.asarray(W, dtype=np.float32)
    b = np.asarray(b, dtype=np.float32)
    gamma = np.asarray(gamma, dtype=np.float32)
    beta = np.asarray(beta, dtype=np.float32)

    WT = np.ascontiguousarray(W.T)
    bgb = np.ascontiguousarray(
        np.broadcast_to(np.concatenate([b, gamma, beta])[None, :], (128, 3 * C))
    )
    ident = np.eye(128, dtype=np.float32)

    in_maps = []
    for k in range(NCORES):
        bb, half = divmod(k, 2)
        i0 = half * ROWS
        # roll rows so this core's i-rows come first; host un-rolls j
        hloc = np.roll(h[bb], -i0, axis=0)
        in_maps.append({
            "hT": np.ascontiguousarray(hloc.T),
            "WT": WT,
            "bgb": bgb,
            "identity": ident,
        })
    return in_maps


def run(h, W, b, gamma, beta, trace=False, **trace_kwargs):
    if "nc" not in _CACHE:
        _CACHE["nc"] = _build_program()
    nc = _CACHE["nc"]
    res = run_bass_kernel_spmd(
        nc,
        _in_maps(h, W, b, gamma, beta),
        core_ids=list(range(NCORES)),
        trace=trace,
        **trace_kwargs,
    )
    out = np.zeros((B, N, N, C), dtype=np.float32)
    for k in range(NCORES):
        bb, half = divmod(k, 2)
        i0 = half * ROWS
        buf = np.asarray(res.results[k]["out"]).astype(np.float32)
        # [ROWS, jh, c, j] -> [ROWS, (jh j), c], then un-roll j
        buf = buf.reshape(ROWS, 2, C, JW).transpose(0, 1, 3, 2).reshape(ROWS, N, C)
        out[bb, i0:i0 + ROWS] = np.roll(buf, i0, axis=1)
    return out, res


def kernel(h, W, b, gamma, beta):
    out, _ = run(h, W, b, gamma, beta)
    return out


# revision 13
# speedup vs baseline: 2.9866x; 1.2480x over previous
"""Trainium2 Bass kernel for nn_DistanceModule.

Computes, for h [4,512,64], W [64,64], b/gamma/beta [64]:
    x = LayerNorm(ReLU(h @ W.T + b))          # [B,N,C]
    D[b,i,j,c] = x[b,i,c] * x[b,j,c]
    out = softmax(D, axis=-1)                 # [B,N,N,C] f32 (256 MB)

Sharding + symmetry: out[b,i,j,c] == out[b,j,i,c] exactly (the product
commutes), so of each batch's 4x4 grid of 128x128 (i,j) blocks only 12
need computing. Core pair (2b, 2b+1): even core takes tile order
(0,1,2,3), odd core (3,2,1,0) -- the SAME program in local tile
coords computes units [(it0,j 0:256), (it0,j 256:512), (it1,j 128:384)]
on both, which lands on blocks rows 0-1 (even) / rows 3-2 (odd).
The host places 12 blocks directly and 4 as transposes.

Per-core pipeline, c-major on-chip layout (contiguous APs everywhere):
  PE     : K=1 outer-product bf16 matmuls (lhsT = xTcat[0, c*N+i0]
           [1,128], rhs = xTcat[0, c*N+j0] [1,512|256]) write logit
           x_i[c]*x_j[c] into PSUM. All x rows are concatenated on
           partition 0 (matmul base-partition rule) via a DRAM bounce.
  ScalarE: contiguous exp activations (FD=1024, PSUM src, bf16 dst).
           Only Ln/Exp are used on ScalarE (single table set).
  VectorE: channel sums via bf16 add-trees in 16-channel partial
           groups (tensor_tensor 2x 16-bit mode), emitted as the exp
           tiles fill so VectorE overlaps the fill; then
           reciprocal_approx_fast and an in-place bf16 normalize
           multiply against a stride-0-broadcast reciprocal.
  DMA    : contiguous bf16 stores in (c,j) order; the host transposes
           blocks to (j,c) while casting back to f32.

Softmax needs no max-subtraction: LayerNorm bounds |x| by sqrt(C-1),
logits <= 63, exp <= 2.4e27 which fits bf16 range.
Measured rel err ~5e-3 vs f32 reference (harness gate 2e-2).
"""

import numpy as np

import concourse.bacc as bacc
import concourse.bass as bass
import concourse.mybir as mybir
import concourse.tile as tile
from concourse.bass_utils import run_bass_kernel_spmd

B, N, C = 4, 512, 64
NCORES = 8
ROWS = 256
JW = 256            # unit j-width
NU = 3              # units per core
EPS = 1e-5
F32 = mybir.dt.float32
BF16 = mybir.dt.bfloat16

_CACHE = {}


def _build_program():
    nc = bacc.Bacc(
        "TRN2",
        target_bir_lowering=False,
        debug=False,
        enable_asserts=False,
        num_devices=NCORES,
    )

    hT_d = nc.dram_tensor("hT", [C, N], F32, kind="ExternalInput")
    WT_d = nc.dram_tensor("WT", [C, C], F32, kind="ExternalInput")
    bgb_d = nc.dram_tensor("bgb", [128, 3 * C], F32, kind="ExternalInput")
    id_d = nc.dram_tensor("identity", [128, 128], F32, kind="ExternalInput")
    xstage_d = nc.dram_tensor("xstage", [C, N], BF16, kind="Internal")
    # unit u -> rows [u*128,(u+1)*128), (c,j)-major columns
    out_d = nc.dram_tensor("out", [NU * 128, C * JW], BF16, kind="ExternalOutput")

    sub = mybir.AluOpType.subtract
    mult = mybir.AluOpType.mult
    Exp = mybir.ActivationFunctionType.Exp
    Ln = mybir.ActivationFunctionType.Ln

    with tile.TileContext(nc) as tc:
        with tc.tile_pool(name="const", bufs=1) as constp:
            hT = constp.tile([C, N], F32)
            nc.sync.dma_start(hT[:], hT_d[:])
            WT = constp.tile([C, C], F32)
            nc.sync.dma_start(WT[:], WT_d[:])
            bgb = constp.tile([128, 3 * C], F32)
            nc.sync.dma_start(bgb[:], bgb_d[:])
            ident = constp.tile([128, 128], F32)
            nc.sync.dma_start(ident[:], id_d[:])

            xT = constp.tile([C, N], BF16)
            eps_t = constp.tile([128, 1], F32)
            nc.vector.memset(eps_t[:], EPS)

            # ---- x = LayerNorm(ReLU(h @ W.T + b)), transposed to bf16 ----
            # ReLU on VectorE; rstd = exp(-0.5*ln(var+eps)) keeps ScalarE in
            # the natural_log_exp table set (one ACT_TABLE_LOAD).
            with (
                tc.tile_pool(name="xprep", bufs=2) as xprep,
                tc.tile_pool(name="psum_prep", bufs=2, space=bass.MemorySpace.PSUM) as psp,
            ):
                for t in range(4):
                    xp = psp.tile([128, C], F32, tag="xp")
                    nc.tensor.matmul(xp[:], hT[:, t * 128:(t + 1) * 128], WT[:])
                    xs = xprep.tile([128, C], F32, tag="xs")
                    nc.vector.tensor_add(xs[:], xp[:], bgb[:, 0:C])      # + b
                    nc.vector.tensor_scalar_max(xs[:], xs[:], 0.0)       # ReLU
                    stats = xprep.tile([128, 6], F32, tag="stats")
                    nc.vector.bn_stats(stats[:], xs[:])
                    mv = xprep.tile([128, 2], F32, tag="mv")
                    nc.vector.bn_aggr(mv[:], stats[:])
                    lnv = xprep.tile([128, 1], F32, tag="lnv")
                    nc.scalar.activation(lnv[:], mv[:, 1:2], Ln, bias=eps_t[:, 0:1])
                    rstd = xprep.tile([128, 1], F32, tag="rstd")
                    nc.scalar.activation(rstd[:], lnv[:], Exp, scale=-0.5)
                    xn = xprep.tile([128, C], F32, tag="xn")
                    nc.vector.tensor_scalar(
                        xn[:], xs[:], mv[:, 0:1], rstd[:, 0:1], op0=sub, op1=mult
                    )
                    nc.vector.tensor_mul(xn[:], xn[:], bgb[:, C:2 * C])  # * gamma
                    nc.vector.tensor_add(xn[:], xn[:], bgb[:, 2 * C:3 * C])  # + beta
                    tp = psp.tile([C, 128], F32, tag="tp")
                    nc.tensor.transpose(tp[:], xn[:], ident[:])
                    nc.vector.tensor_copy(xT[:, t * 128:(t + 1) * 128], tp[:])

            # concatenate all xT rows onto partition 0 (matmul operands must
            # have base partition 0/32/64): SBUF -> DRAM -> SBUF bounce
            xTcat = constp.tile([1, C * N], BF16)
            nc.sync.dma_start(xstage_d[:], xT[:])
            nc.sync.dma_start(
                xTcat[0:1, :], xstage_d[:].rearrange("a b -> (a b)")[None, :]
            )

            # ---- main: exp(x_i*x_j), softmax over c, store ----------------
            with (
                tc.tile_pool(name="main", bufs=3) as mainp,
                tc.tile_pool(name="scr", bufs=3) as scrp,
                tc.tile_pool(name="small", bufs=6) as smallp,
                tc.tile_pool(name="psum_bc", bufs=2, space=bass.MemorySpace.PSUM) as pbc,
            ):
                units = []  # (expt, sc) per unit

                def new_unit():
                    expt = mainp.tile([128, C, JW], BF16, tag="exp")
                    sc = scrp.tile([128, 4, JW], BF16, tag="sc")
                    units.append((expt, sc))
                    return units[-1]

                def partial_tree(expt, sc, g):
                    """Sum channels [16g,16g+16) of expt into sc[:, g, :]."""
                    e = expt[:, 16 * g:16 * (g + 1), :]
                    s8 = scrp.tile([128, 8, JW], BF16, tag="s8")
                    nc.vector.tensor_add(s8[:], e[:, 0:8, :], e[:, 8:16, :])
                    nc.vector.tensor_add(s8[:, 0:4, :], s8[:, 0:4, :], s8[:, 4:8, :])
                    nc.vector.tensor_add(s8[:, 0:2, :], s8[:, 0:2, :], s8[:, 2:4, :])
                    nc.vector.tensor_add(sc[:, g, :], s8[:, 0, :], s8[:, 1, :])

                def finish_unit(u):
                    """Combine partials, recip, normalize, store unit u."""
                    expt, sc = units[u]
                    s2 = smallp.tile([128, 2, JW], BF16, tag="s2")
                    nc.vector.tensor_add(s2[:], sc[:, 0:2, :], sc[:, 2:4, :])
                    s32 = smallp.tile([128, JW], F32, tag="s32")
                    nc.vector.tensor_add(s32[:], s2[:, 0, :], s2[:, 1, :])
                    recip = smallp.tile([128, JW], F32, tag="recip")
                    nc.vector.reciprocal_approx_fast(recip[:], s32[:])
                    rb = smallp.tile([128, JW], BF16, tag="rb")
                    nc.vector.tensor_copy(rb[:], recip[:])
                    CC = 16
                    for q in range(C // CC):
                        cs = slice(q * CC, (q + 1) * CC)
                        nc.vector.tensor_mul(
                            expt[:, cs, :],
                            expt[:, cs, :],
                            rb[:, None, :].broadcast_to((128, CC, JW)),
                        )
                        nc.sync.dma_start(
                            out_d[u * 128:(u + 1) * 128,
                                  q * CC * JW:(q + 1) * CC * JW],
                            expt[:, cs, :].rearrange("p c j -> p (c j)"),
                        )

                # --- i-tile 0: units 0 (j 0:256) and 1 (j 256:512) share
                # 512-col matmul rounds; dual half-activations ---
                e0, sc0 = new_unit()
                e1, sc1 = new_unit()
                for r in range(16):
                    ps = pbc.tile([128, 4, 2 * JW], F32, tag="ps")
                    for ci in range(4):
                        c = 4 * r + ci
                        nc.tensor.matmul(
                            ps[:, ci, :],
                            xTcat[0:1, c * N:c * N + 128],
                            xTcat[0:1, c * N:c * N + N],
                        )
                    nc.scalar.activation(
                        e0[:, 4 * r:4 * (r + 1), :], ps[:, :, 0:JW], Exp
                    )
                    nc.scalar.activation(
                        e1[:, 4 * r:4 * (r + 1), :], ps[:, :, JW:2 * JW], Exp
                    )
                    if r % 4 == 3:
                        partial_tree(e0, sc0, r // 4)
                        partial_tree(e1, sc1, r // 4)

                # --- i-tile 1: unit 2 (j 128:384), 256-col matmuls ---
                e2, sc2 = new_unit()
                for r in range(16):
                    ps = pbc.tile([128, 4, 2 * JW], F32, tag="ps")
                    for ci in range(4):
                        c = 4 * r + ci
                        nc.tensor.matmul(
                            ps[:, ci, JW // 2:3 * JW // 2],
                            xTcat[0:1, c * N + 128:c * N + 256],
                            xTcat[0:1, c * N + 128:c * N + 384],
                        )
                    nc.scalar.activation(
                        e2[:, 4 * r:4 * (r + 1), :],
                        ps[:, :, JW // 2:3 * JW // 2], Exp
                    )
                    if r % 4 == 3:
                        partial_tree(e2, sc2, r // 4)
                    if r == 1:
                        finish_unit(0)
                    if r == 9:
                        finish_unit(1)
                finish_unit(2)
    nc.compile()
    return nc


def _in_maps(h, W, b, gamma, beta):
    h = np.asarray(h, dtype=np.float32)
    W = np.asarray(W, dtype=np.float32)
    b = np.asarray(b, dtype=np.float32)
    gamma = np.asarray(gamma, dtype=np.float32)
    beta = np.asarray(beta, dtype=np.float32)

    WT = np.ascontiguousarray(W.T)
    bgb = np.ascontiguousarray(
        np.broadcast_to(np.concatenate([b, gamma, beta])[None, :], (128, 3 * C))
    )
    ident = np.eye(128, dtype=np.float32)

    in_maps = []
    for k in range(NCORES):
        bb, half = divmod(k, 2)
        hloc = h[bb]
        if half == 1:  # odd core: tile order (3,2,1,0)
            hloc = hloc.reshape(4, 128, C)[::-1].reshape(N, C)
        in_maps.append({
            "hT": np.ascontiguousarray(hloc.T),
            "WT": WT,
            "bgb": bgb,
            "identity": ident,
        })
    return in_maps


def run(h, W, b, gamma, beta, trace=False, **trace_kwargs):
    if "nc" not in _CACHE:
        _CACHE["nc"] = _build_program()
    nc = _CACHE["nc"]
    res = run_bass_kernel_spmd(
        nc,
        _in_maps(h, W, b, gamma, beta),
        core_ids=list(range(NCORES)),
        trace=trace,
        **trace_kwargs,
    )
    out = np.zeros((B, N, N, C), dtype=np.float32)
    for bb in range(B):
        blocks = {}
        for half in (0, 1):
            buf = np.asarray(res.results[2 * bb + half]["out"]).astype(np.float32)
            # [3u, 128i, C, JW] -> [3u, 128i, JW j, C]
            arr = buf.reshape(NU, 128, C, JW).transpose(0, 1, 3, 2)
            # global (i-tile, j-tile) of each unit's two 128-j halves
            if half == 0:
                tiles = [(0, 0), (0, 1), (0, 2), (0, 3), (1, 1), (1, 2)]
            else:
                tiles = [(3, 3), (3, 2), (3, 1), (3, 0), (2, 2), (2, 1)]
            for u in range(NU):
                for s in range(2):
                    blocks[tiles[2 * u + s]] = arr[u][:, 128 * s:128 * (s + 1)]
        # symmetric completions
        blocks[(1, 0)] = blocks[(0, 1)].transpose(1, 0, 2)
        blocks[(2, 0)] = blocks[(0, 2)].transpose(1, 0, 2)
        blocks[(1, 3)] = blocks[(3, 1)].transpose(1, 0, 2)
        blocks[(2, 3)] = blocks[(3, 2)].transpose(1, 0, 2)
        for (ti, tj), blk in blocks.items():
            out[bb, 128 * ti:128 * (ti + 1), 128 * tj:128 * (tj + 1)] = blk
    return out, res


def kernel(h, W, b, gamma, beta):
    out, _ = run(h, W, b, gamma, beta)
    return out


# revision 18
# speedup vs baseline: 3.2251x; 1.0799x over previous
"""Trainium2 Bass kernel for nn_DistanceModule.

Computes, for h [4,512,64], W [64,64], b/gamma/beta [64]:
    x = LayerNorm(ReLU(h @ W.T + b))          # [B,N,C]
    D[b,i,j,c] = x[b,i,c] * x[b,j,c]
    out = softmax(D, axis=-1)                 # [B,N,N,C] f32 (256 MB)

Sharding + symmetry: out[b,i,j,c] == out[b,j,i,c] exactly (the product
commutes), so of each batch's 4x4 grid of 128x128 (i,j) blocks only 12
need computing. Core pair (2b, 2b+1): even core takes tile order
(0,1,2,3), odd core (3,2,1,0) -- the SAME program in local tile
coords computes units [(it0,j 0:256), (it0,j 256:512), (it1,j 128:384)]
on both, which lands on blocks rows 0-1 (even) / rows 3-2 (odd).
The host places 12 blocks directly and 4 as transposes.

Per-core pipeline, c-major on-chip layout (contiguous APs everywhere):
  PE     : K=1 outer-product bf16 matmuls (lhsT = xTcat[0, c*N+i0]
           [1,128], rhs = xTcat[0, c*N+j0] [1,512|256]) write logit
           x_i[c]*x_j[c] into PSUM. All x rows are concatenated on
           partition 0 (matmul base-partition rule) via a DRAM bounce.
  ScalarE: contiguous exp activations (FD=1024, PSUM src, bf16 dst).
           Only Ln/Exp are used on ScalarE (single table set).
  VectorE: channel sums via bf16 add-trees in 16-channel partial
           groups (tensor_tensor 2x 16-bit mode), emitted as the exp
           tiles fill so VectorE overlaps the fill; then
           reciprocal_approx_fast and an in-place bf16 normalize
           multiply against a stride-0-broadcast reciprocal.
  DMA    : contiguous bf16 stores in (c,j) order; the host transposes
           blocks to (j,c) while casting back to f32.

Softmax needs no max-subtraction: LayerNorm bounds |x| by sqrt(C-1),
logits <= 63, exp <= 2.4e27 which fits bf16 range.
Measured rel err ~5e-3 vs f32 reference (harness gate 2e-2).
"""

import numpy as np

import concourse.bacc as bacc
import concourse.bass as bass
import concourse.mybir as mybir
import concourse.tile as tile
from concourse.bass_utils import run_bass_kernel_spmd

B, N, C = 4, 512, 64
NCORES = 8
ROWS = 256
JW = 256            # unit j-width
NU = 3              # units per core
EPS = 1e-5
F32 = mybir.dt.float32
BF16 = mybir.dt.bfloat16

_CACHE = {}


def _build_program():
    nc = bacc.Bacc(
        "TRN2",
        target_bir_lowering=False,
        debug=False,
        enable_asserts=False,
        num_devices=NCORES,
    )

    hT_d = nc.dram_tensor("hT", [C, N], F32, kind="ExternalInput")
    WT_d = nc.dram_tensor("WT", [C, C], F32, kind="ExternalInput")
    bgb_d = nc.dram_tensor("bgb", [128, 3 * C], F32, kind="ExternalInput")
    id_d = nc.dram_tensor("identity", [128, 128], F32, kind="ExternalInput")
    xstage_d = nc.dram_tensor("xstage", [C, N], BF16, kind="Internal")
    # unit u -> rows [u*128,(u+1)*128), (c,j)-major columns
    out_d = nc.dram_tensor("out", [NU * 128, C * JW], BF16, kind="ExternalOutput")

    sub = mybir.AluOpType.subtract
    mult = mybir.AluOpType.mult
    Exp = mybir.ActivationFunctionType.Exp
    Ln = mybir.ActivationFunctionType.Ln

    with tile.TileContext(nc) as tc:
        with tc.tile_pool(name="const", bufs=1) as constp:
            hT = constp.tile([C, N], F32)
            nc.sync.dma_start(hT[:], hT_d[:])
            WT = constp.tile([C, C], F32)
            nc.sync.dma_start(WT[:], WT_d[:])
            bgb = constp.tile([128, 3 * C], F32)
            nc.sync.dma_start(bgb[:], bgb_d[:])
            ident = constp.tile([128, 128], F32)
            nc.sync.dma_start(ident[:], id_d[:])

            xT = constp.tile([C, N], BF16)
            eps_t = constp.tile([128, 1], F32)
            nc.vector.memset(eps_t[:], EPS)

            # ---- x = LayerNorm(ReLU(h @ W.T + b)), transposed to bf16 ----
            # ReLU on VectorE; rstd = exp(-0.5*ln(var+eps)) keeps ScalarE in
            # the natural_log_exp table set (one ACT_TABLE_LOAD).
            with (
                tc.tile_pool(name="xprep", bufs=2) as xprep,
                tc.tile_pool(name="psum_prep", bufs=2, space=bass.MemorySpace.PSUM) as psp,
            ):
                # PE HAM warmup: ~5us of back-to-back dummy matmuls while
                # the input DMAs land, so the clock gate reaches 8/8 before
                # the product matmuls start (PE would otherwise sit at half
                # clock all kernel: its bursts are shorter than the 4us the
                # HAM needs).
                warm = psp.tile([128, 128], F32, tag="warm")
                for _ in range(12):
                    nc.tensor.matmul(warm[:], ident[:, 0:128], ident[:])
                for t in range(4):
                    xp = psp.tile([128, C], F32, tag="xp")
                    nc.tensor.matmul(xp[:], hT[:, t * 128:(t + 1) * 128], WT[:])
                    xs = xprep.tile([128, C], F32, tag="xs")
                    nc.vector.tensor_add(xs[:], xp[:], bgb[:, 0:C])      # + b
                    nc.vector.tensor_scalar_max(xs[:], xs[:], 0.0)       # ReLU
                    stats = xprep.tile([128, 6], F32, tag="stats")
                    nc.vector.bn_stats(stats[:], xs[:])
                    mv = xprep.tile([128, 2], F32, tag="mv")
                    nc.vector.bn_aggr(mv[:], stats[:])
                    lnv = xprep.tile([128, 1], F32, tag="lnv")
                    nc.scalar.activation(lnv[:], mv[:, 1:2], Ln, bias=eps_t[:, 0:1])
                    rstd = xprep.tile([128, 1], F32, tag="rstd")
                    nc.scalar.activation(rstd[:], lnv[:], Exp, scale=-0.5)
                    xn = xprep.tile([128, C], F32, tag="xn")
                    nc.vector.tensor_scalar(
                        xn[:], xs[:], mv[:, 0:1], rstd[:, 0:1], op0=sub, op1=mult
                    )
                    nc.vector.tensor_mul(xn[:], xn[:], bgb[:, C:2 * C])  # * gamma
                    nc.vector.tensor_add(xn[:], xn[:], bgb[:, 2 * C:3 * C])  # + beta
                    tp = psp.tile([C, 128], F32, tag="tp")
                    nc.tensor.transpose(tp[:], xn[:], ident[:])
                    nc.vector.tensor_copy(xT[:, t * 128:(t + 1) * 128], tp[:])

            # concatenate all xT rows onto partition 0 (matmul operands must
            # have base partition 0/32/64): SBUF -> DRAM -> SBUF bounce
            xTcat = constp.tile([1, C * N], BF16)
            nc.sync.dma_start(xstage_d[:], xT[:])
            nc.sync.dma_start(
                xTcat[0:1, :], xstage_d[:].rearrange("a b -> (a b)")[None, :]
            )
            # keep the PE busy across the bounce (a >3us idle would drop
            # the HAM clock gate back to half rate)
            with tc.tile_pool(name="psum_w", bufs=1, space=bass.MemorySpace.PSUM) as psw:
                warm2 = psw.tile([128, 128], F32, tag="warm2")
                for _ in range(10):
                    nc.tensor.matmul(warm2[:], ident[:, 0:128], ident[:])

            # ---- main: exp(x_i*x_j), softmax over c, store ----------------
            with (
                tc.tile_pool(name="main", bufs=1) as mainp,
                tc.tile_pool(name="scr", bufs=2) as scrp,
                tc.tile_pool(name="small", bufs=2) as smallp,
                tc.tile_pool(name="psum_bc", bufs=2, space=bass.MemorySpace.PSUM) as pbc,
            ):
                def partial_tree(expt, sc, g, jw):
                    """Sum channels [8g,8g+8) of expt into sc[:, g, 0:jw]."""
                    e = expt[:, 8 * g:8 * (g + 1), 0:jw]
                    s4 = scrp.tile([128, 4, N], BF16, tag="s4")
                    nc.vector.tensor_add(s4[:, :, 0:jw], e[:, 0:4, :], e[:, 4:8, :])
                    nc.vector.tensor_add(
                        s4[:, 0:2, 0:jw], s4[:, 0:2, 0:jw], s4[:, 2:4, 0:jw]
                    )
                    nc.vector.tensor_add(
                        sc[:, g, 0:jw], s4[:, 0, 0:jw], s4[:, 1, 0:jw]
                    )

                def finish(expt, sc, jw, rows):
                    """Combine partials, recip, normalize, store.
                    rows = list of (out_row0, j0) 128-j store groups."""
                    nc.vector.tensor_add(
                        sc[:, 0:4, 0:jw], sc[:, 0:4, 0:jw], sc[:, 4:8, 0:jw]
                    )
                    nc.vector.tensor_add(
                        sc[:, 0:2, 0:jw], sc[:, 0:2, 0:jw], sc[:, 2:4, 0:jw]
                    )
                    s32 = smallp.tile([128, N], F32, tag="s32")
                    nc.vector.tensor_add(
                        s32[:, 0:jw], sc[:, 0, 0:jw], sc[:, 1, 0:jw]
                    )
                    recip = smallp.tile([128, N], F32, tag="recip")
                    nc.vector.reciprocal_approx_fast(recip[:, 0:jw], s32[:, 0:jw])
                    rb = smallp.tile([128, N], BF16, tag="rb")
                    nc.vector.tensor_copy(rb[:, 0:jw], recip[:, 0:jw])
                    CC = 8
                    for q in range(C // CC):
                        cs = slice(q * CC, (q + 1) * CC)
                        nc.vector.tensor_mul(
                            expt[:, cs, 0:jw],
                            expt[:, cs, 0:jw],
                            rb[:, None, 0:jw].broadcast_to((128, CC, jw)),
                        )
                        for row0, j0 in rows:
                            nc.sync.dma_start(
                                out_d[row0:row0 + 128,
                                      q * CC * JW:(q + 1) * CC * JW]
                                .rearrange("p (c j) -> p c j", j=JW),
                                expt[:, cs, j0:j0 + JW],
                            )

                # --- i-tile 0: one [128, C, 512] tile covers units 0+1 ---
                e01 = mainp.tile([128, C, N], BF16, tag="exp01")
                sc01 = scrp.tile([128, 8, N], BF16, tag="sc")
                for r in range(16):
                    ps = pbc.tile([128, 4, N], F32, tag="ps")
                    for ci in range(4):
                        c = 4 * r + ci
                        nc.tensor.matmul(
                            ps[:, ci, :],
                            xTcat[0:1, c * N:c * N + 128],
                            xTcat[0:1, c * N:c * N + N],
                        )
                    nc.scalar.activation(e01[:, 4 * r:4 * (r + 1), :], ps[:], Exp)
                    if r % 2 == 1:
                        partial_tree(e01, sc01, r // 2, N)

                # all of finish01's vector work is ready at i-tile-0 end;
                # emit it BEFORE unit 2's partials so the vector queue never
                # head-of-line blocks on unit 2's activations
                fin01 = lambda: finish(e01, sc01, N, [(0, 0), (128, JW)])

                # --- i-tile 1: unit 2 (j 128:384), 8-channel rounds ---
                e2 = mainp.tile([128, C, JW], BF16, tag="exp2")
                sc2 = scrp.tile([128, 8, N], BF16, tag="sc")
                for r in range(8):
                    ps = pbc.tile([128, 8, JW], F32, tag="ps")
                    for ci in range(8):
                        c = 8 * r + ci
                        nc.tensor.matmul(
                            ps[:, ci, :],
                            xTcat[0:1, c * N + 128:c * N + 256],
                            xTcat[0:1, c * N + 128:c * N + 384],
                        )
                    nc.scalar.activation(e2[:, 8 * r:8 * (r + 1), :], ps[:], Exp)
                    if r == 0:
                        fin01()

                for g in range(8):
                    partial_tree(e2, sc2, g, JW)
                finish(e2, sc2, JW, [(256, 0)])
    nc.compile()
    return nc


def _in_maps(h, W, b, gamma, beta):
    h = np.asarray(h, dtype=np.float32)
    W = np.asarray(W, dtype=np.float32)
    b = np.asarray(b, dtype=np.float32)
    gamma = np.asarray(gamma, dtype=np.float32)
    beta = np.asarray(beta, dtype=np.float32)

    WT = np.ascontiguousarray(W.T)
    bgb = np.ascontiguousarray(
        np.broadcast_to(np.concatenate([b, gamma, beta])[None, :], (128, 3 * C))
    )
    ident = np.eye(128, dtype=np.float32)

    in_maps = []
    for k in range(NCORES):
        bb, half = divmod(k, 2)
        hloc = h[bb]
        if half == 1:  # odd core: tile order (3,2,1,0)
            hloc = hloc.reshape(4, 128, C)[::-1].reshape(N, C)
        in_maps.append({
            "hT": np.ascontiguousarray(hloc.T),
            "WT": WT,
            "bgb": bgb,
            "identity": ident,
        })
    return in_maps


def run(h, W, b, gamma, beta, trace=False, **trace_kwargs):
    if "nc" not in _CACHE:
        _CACHE["nc"] = _build_program()
    nc = _CACHE["nc"]
    res = run_bass_kernel_spmd(
        nc,
        _in_maps(h, W, b, gamma, beta),
        core_ids=list(range(NCORES)),
        trace=trace,
        **trace_kwargs,
    )
    out = np.zeros((B, N, N, C), dtype=np.float32)
    for bb in range(B):
        blocks = {}
        for half in (0, 1):
            buf = np.asarray(res.results[2 * bb + half]["out"]).astype(np.float32)
            # [3u, 128i, C, JW] -> [3u, 128i, JW j, C]
            arr = buf.reshape(NU, 128, C, JW).transpose(0, 1, 3, 2)
            # global (i-tile, j-tile) of each unit's two 128-j halves
            if half == 0:
                tiles = [(0, 0), (0, 1), (0, 2), (0, 3), (1, 1), (1, 2)]
            else:
                tiles = [(3, 3), (3, 2), (3, 1), (3, 0), (2, 2), (2, 1)]
            for u in range(NU):
                for s in range(2):
                    blocks[tiles[2 * u + s]] = arr[u][:, 128 * s:128 * (s + 1)]
        # symmetric completions
        blocks[(1, 0)] = blocks[(0, 1)].transpose(1, 0, 2)
        blocks[(2, 0)] = blocks[(0, 2)].transpose(1, 0, 2)
        blocks[(1, 3)] = blocks[(3, 1)].transpose(1, 0, 2)
        blocks[(2, 3)] = blocks[(3, 2)].transpose(1, 0, 2)
        for (ti, tj), blk in blocks.items():
            out[bb, 128 * ti:128 * (ti + 1), 128 * tj:128 * (tj + 1)] = blk
    return out, res


def kernel(h, W, b, gamma, beta):
    out, _ = run(h, W, b, gamma, beta)
    return out
